# revision 14
# baseline (speedup 1.0000x reference)
"""Trainium2 Bass kernel for the DeepBayesianFilterBlockDiag loss.

Strategy (8-core SPMD, observation-axis sharded):
  - The 152064-dim observation axis is split into 8 shards of 19008 columns.
    Each core gets its shard of target [256,19008], W_dec||b_dec [65,19008],
    log_R [19008], plus the full (tiny) per-(b,t,z) tensors.
  - Per core:
      * compute Xe = [mu_f + chol(sigma_f) @ eps, 1] (tiny 2x2 algebra),
        transposed+negated into a [65,256] lhsT.
      * main loop over column chunks: PE "injects" the target into PSUM via an
        identity matmul, then accumulates -Xe @ W' on top, leaving
        d = t - rec in PSUM with zero DVE work.  ACT squares d into SBUF,
        PE reduces columns (ones-lhsT matmul) into a per-chunk row of a
        PSUM accumulator.
      * epilogue: weight column sums by exp(-2 log_R) (DVE fused
        tensor_tensor_reduce), reduce log_R, reduce the KL partials, and
        emit a [3] vector (sse, sum_logR, kl_raw).
  - Host combines the 8 partial vectors into the final scalar loss.
"""

import math

import numpy as np

import concourse.bass as bass
import concourse.mybir as mybir
import concourse.tile as tile
from concourse.bass_utils import run_bass_kernel_spmd
from concourse.masks import make_identity

F32 = mybir.dt.float32
AF = mybir.ActivationFunctionType
OP = mybir.AluOpType

B, T, Z, DIM = 4, 64, 32, 2
ROWS = B * T          # 256
LAT = Z * DIM         # 64
LATP = LAT + 1        # 65 (ones row folds in b_dec)
D_OBS = 152064
NCORES = 8
DC = D_OBS // NCORES  # 19008 columns per core
CH = 512              # psum-bank column chunk
GRP = 1024            # ACT / psum group (2 chunks)
N_FULL = DC // CH     # 37 full chunks
REM = DC - N_FULL * CH  # 64

CCH = 128             # colsum chunk (transposed-reduce matmul width)
MAX_DRAIN_WAITS = 1
USE_INJECT = True


def _layout(dc):
    groups = []
    off = 0
    while off < dc:
        g = []
        goff = off
        for _ in range(GRP // CH):
            w = min(CH, dc - off)
            if w <= 0:
                break
            g.append((off - goff, w))
            off += w
        groups.append((goff, g))
    n_full = dc // CH
    rem = dc - n_full * CH
    ncc = (dc + CCH - 1) // CCH
    return groups, n_full, rem, ncc


def _split_multi_waits(nc, max_waits=1):
    """walrus' per-instruction sync encoding only fits one wait; move extra
    waits emitted by Tile onto NOPs inserted just before the instruction on
    the same engine (same semantics: engine blocks on all of them in order).
    """
    k = 0
    for f in nc.m.functions:
        for blk in f.blocks:
            il = blk.instructions
            i = 0
            while i < len(il):
                inst = il[i]
                si = inst.sync_info
                if si is not None and len(si.on_wait) > max_waits:
                    waits = list(si.on_wait)
                    inst.sync_info = mybir.SyncInfo(
                        on_wait=waits[-max_waits:], on_update=list(si.on_update)
                    )
                    extra = waits[:-max_waits]
                    for j in range(0, len(extra), max_waits):
                        nop = mybir.InstEventSemaphore(
                            name=f"{inst.name}-w{k}",
                            engine=inst.engine,
                            sync_info=mybir.SyncInfo(
                                on_wait=extra[j : j + max_waits], on_update=[]
                            ),
                        )
                        k += 1
                        il.insert(i, nop)
                        i += 1
                i += 1


def _comp4(t, mg, idx):
    # [128, 2, 128] tile -> [128, 32] view of 2x2-block component idx
    return t[:, mg, :].rearrange("p (z k) -> p z k", k=4)[:, :, idx]


def _comp2(t, mg, idx):
    return t[:, mg, :].rearrange("p (z k) -> p z k", k=2)[:, :, idx]


def build_nc(reps: int = 1, dc: int = DC, split_waits: bool = True):
    nc = bass.Bass("TRN2")
    tgt = nc.dram_tensor("tgt", [ROWS, dc], F32, kind="ExternalInput")
    wb = nc.dram_tensor("wb", [LATP, dc], F32, kind="ExternalInput")
    lr = nc.dram_tensor("log_r", [dc], F32, kind="ExternalInput")
    muf = nc.dram_tensor("mu_f", [ROWS, LAT], F32, kind="ExternalInput")
    sgf = nc.dram_tensor("sig_f", [ROWS, 4 * Z], F32, kind="ExternalInput")
    mup = nc.dram_tensor("mu_p", [ROWS, LAT], F32, kind="ExternalInput")
    sgp = nc.dram_tensor("sig_p", [ROWS, 4 * Z], F32, kind="ExternalInput")
    eps = nc.dram_tensor("eps", [ROWS, LAT], F32, kind="ExternalInput")
    out = nc.dram_tensor("out", [5], F32, kind="ExternalOutput")

    with tile.TileContext(nc) as tc:
        with (
            tc.tile_pool(name="big", bufs=1) as big,
            tc.tile_pool(name="tp", bufs=4) as tpool,
            tc.tile_pool(name="sp", bufs=3) as spool,
            tc.tile_pool(name="small", bufs=1) as small,
            tc.tile_pool(name="dps", bufs=2, space="PSUM") as dpsum,
            tc.tile_pool(name="acc", bufs=1, space="PSUM") as accpsum,
            tc.tile_pool(name="smallps", bufs=1, space="PSUM") as smallps,
        ):
            for _ in range(reps):
                _body(nc, tc, big, tpool, spool, small, dpsum, accpsum, smallps,
                      tgt, wb, lr, muf, sgf, mup, sgp, eps, out, dc)
    if split_waits:
        # needed for the walrus/HW path; CoreSim wants the raw form
        _split_multi_waits(nc)
    return nc


def _body(nc, tc, big, tpool, spool, small, dpsum, accpsum, smallps,
          tgt, wb, lr, muf, sgf, mup, sgp, eps, out, dc=DC):
    GROUPS, N_FULL, REM, NCC = _layout(dc)
    DCL = dc
    ident = small.tile([128, 128], F32)
    make_identity(nc, ident)
    ones = small.tile([128, 1], F32)
    nc.vector.memset(ones, 1.0)

    # ---- small inputs ----
    sigf_s = small.tile([128, 2, 4 * Z], F32)
    sigp_s = small.tile([128, 2, 4 * Z], F32)
    muf_s = small.tile([128, 2, LAT], F32)
    mup_s = small.tile([128, 2, LAT], F32)
    eps_s = small.tile([128, 2, LAT], F32)
    for mg in range(2):
        rs = slice(mg * 128, (mg + 1) * 128)
        nc.gpsimd.dma_start(out=sigf_s[:, mg, :], in_=sgf[rs, :])
        nc.gpsimd.dma_start(out=sigp_s[:, mg, :], in_=sgp[rs, :])
        nc.gpsimd.dma_start(out=muf_s[:, mg, :], in_=muf[rs, :])
        nc.gpsimd.dma_start(out=mup_s[:, mg, :], in_=mup[rs, :])
        nc.gpsimd.dma_start(out=eps_s[:, mg, :], in_=eps[rs, :])

    lr37 = small.tile([N_FULL, CH], F32)
    lrrem = small.tile([1, REM], F32)
    nc.gpsimd.dma_start(
        out=lr37, in_=lr[0 : N_FULL * CH].rearrange("(p f) -> p f", f=CH)
    )
    nc.gpsimd.dma_start(
        out=lrrem, in_=lr[N_FULL * CH : DCL].rearrange("(p f) -> p f", f=REM)
    )

    # ---- phase 1: Xe (cholesky sample) + KL, per 128-row group ----
    lhsT = small.tile([LATP, 256], F32)
    nc.vector.memset(lhsT[LAT:LATP, :], -1.0)
    kl2 = small.tile([128, 2], F32)

    for mg in range(2):
        af = _comp4(sigf_s, mg, 0)
        bf = _comp4(sigf_s, mg, 1)
        cf = _comp4(sigf_s, mg, 2)
        df = _comp4(sigf_s, mg, 3)
        aq = _comp4(sigp_s, mg, 0)
        bq = _comp4(sigp_s, mg, 1)
        cq = _comp4(sigp_s, mg, 2)
        dq = _comp4(sigp_s, mg, 3)

        # cholesky: l11 = sqrt(a); l21 = c/l11; l22 = sqrt(d - l21^2)
        l11 = small.tile([128, Z], F32)
        nc.scalar.sqrt(l11, af)
        r11 = small.tile([128, Z], F32)
        nc.vector.reciprocal(r11, l11)
        l21 = small.tile([128, Z], F32)
        nc.vector.tensor_mul(l21, cf, r11)
        tmp0 = small.tile([128, Z], F32)
        nc.vector.tensor_mul(tmp0, l21, l21)
        nc.vector.tensor_sub(tmp0, df, tmp0)
        l22 = small.tile([128, Z], F32)
        nc.scalar.sqrt(l22, tmp0)

        e1 = _comp2(eps_s, mg, 0)
        e2 = _comp2(eps_s, mg, 1)
        m1 = _comp2(muf_s, mg, 0)
        m2 = _comp2(muf_s, mg, 1)

        xew = small.tile([128, LAT], F32)
        x1v = xew.rearrange("p (z k) -> p z k", k=2)[:, :, 0]
        x2v = xew.rearrange("p (z k) -> p z k", k=2)[:, :, 1]
        tA = small.tile([128, Z], F32)
        nc.vector.tensor_mul(tA, l11, e1)
        nc.vector.tensor_add(x1v, tA, m1)
        tB = small.tile([128, Z], F32)
        nc.vector.tensor_mul(tB, l21, e1)
        tC = small.tile([128, Z], F32)
        nc.vector.tensor_mul(tC, l22, e2)
        nc.vector.tensor_add(tB, tB, tC)
        nc.vector.tensor_add(x2v, tB, m2)

        tps = smallps.tile([LAT, 128], F32, tag="sps")
        nc.tensor.transpose(tps, xew, ident)
        nc.scalar.mul(lhsT[0:LAT, mg * 128 : (mg + 1) * 128], tps, -1.0)

        # KL pieces
        detq = small.tile([128, Z], F32)
        tD = small.tile([128, Z], F32)
        nc.vector.tensor_mul(detq, aq, dq)
        nc.vector.tensor_mul(tD, bq, cq)
        nc.vector.tensor_sub(detq, detq, tD)
        detp = small.tile([128, Z], F32)
        nc.vector.tensor_mul(detp, af, df)
        nc.vector.tensor_mul(tD, bf, cf)
        nc.vector.tensor_sub(detp, detp, tD)
        rdq = small.tile([128, Z], F32)
        nc.vector.reciprocal(rdq, detq)

        # trace numerator: dq*af - bq*bf - cq*cf + aq*df
        tn = small.tile([128, Z], F32)
        nc.vector.tensor_mul(tn, dq, af)
        nc.vector.tensor_mul(tD, aq, df)
        nc.vector.tensor_add(tn, tn, tD)
        nc.vector.tensor_mul(tD, bq, bf)
        nc.vector.tensor_sub(tn, tn, tD)
        nc.vector.tensor_mul(tD, cq, cf)
        nc.vector.tensor_sub(tn, tn, tD)

        # quad numerator: dq*d1^2 - (bq+cq)*d1*d2 + aq*d2^2
        p1 = _comp2(mup_s, mg, 0)
        p2 = _comp2(mup_s, mg, 1)
        d1 = small.tile([128, Z], F32)
        nc.vector.tensor_sub(d1, p1, m1)
        d2 = small.tile([128, Z], F32)
        nc.vector.tensor_sub(d2, p2, m2)
        qn = small.tile([128, Z], F32)
        nc.vector.tensor_mul(tD, d1, d1)
        nc.vector.tensor_mul(qn, dq, tD)
        nc.vector.tensor_mul(tD, d2, d2)
        nc.vector.tensor_mul(tD, aq, tD)
        nc.vector.tensor_add(qn, qn, tD)
        nc.vector.tensor_mul(tD, d1, d2)
        tE = small.tile([128, Z], F32)
        nc.vector.tensor_add(tE, bq, cq)
        nc.vector.tensor_mul(tD, tD, tE)
        nc.vector.tensor_sub(qn, qn, tD)

        klv = small.tile([128, Z], F32)
        nc.vector.tensor_add(klv, tn, qn)
        nc.vector.tensor_mul(klv, klv, rdq)
        # + ln(detq) - ln(detp)
        nc.scalar.activation(tD, detq, AF.Ln)
        nc.vector.tensor_add(klv, klv, tD)
        nc.scalar.activation(tD, detp, AF.Ln)
        nc.vector.tensor_sub(klv, klv, tD)
        nc.vector.reduce_sum(out=kl2[:, mg : mg + 1], in_=klv, axis=mybir.AxisListType.X)

    # w = exp(-2 log_R) (same ACT table set as Ln)
    w37 = small.tile([N_FULL, CH], F32)
    nc.scalar.activation(w37, lr37, AF.Exp, scale=-2.0)
    wrem = small.tile([1, REM], F32)
    nc.scalar.activation(wrem, lrrem, AF.Exp, scale=-2.0)

    # transpose w into [128, NCC]: wfull[p, cc] = w[cc*128 + p]
    wfull = small.tile([128, (N_FULL + 1) * (CH // CCH)], F32)  # [128, 152]
    nc.vector.memset(wfull, 0.0)
    wview = wfull.rearrange("p (r j) -> p r j", j=CH // CCH)  # [128, 38, 4]
    for j in range(CH // CCH):
        wtp = smallps.tile([128, N_FULL], F32, tag="sps")
        nc.tensor.transpose(wtp, w37[:, j * CCH : (j + 1) * CCH], ident[0:N_FULL, 0:N_FULL])
        nc.scalar.copy(wview[:, 0:N_FULL, j], wtp)
    wtr = smallps.tile([REM, 1], F32, tag="sps")
    nc.tensor.transpose(wtr, wrem, ident[0:1, 0:1])
    nc.scalar.copy(wfull[0:REM, NCC - 1 : NCC], wtr)

    # ---- W' (with b_dec row) resident in SBUF, loaded per-group ----
    wb_s = big.tile([LATP, DCL], F32)

    # colsum bank: column-sums of squares land on partitions.
    # mg0 -> free slots [0, NCC), mg1 -> [256, 256+NCC)
    colsum = accpsum.tile([128, 512], F32)
    nc.vector.memset(colsum, 0.0)

    # ---- phase 2: main loop ----
    first_mg = True
    for mg in range(2):
        rs = slice(mg * 128, (mg + 1) * 128)
        lhsT_mg = lhsT[:, mg * 128 : (mg + 1) * 128]
        for goff, chunks in GROUPS:
            gw = sum(w for _, w in chunks)
            t_s = tpool.tile([128, GRP], F32)
            nc.sync.dma_start(out=t_s[:, 0:gw], in_=tgt[rs, goff : goff + gw])
            if first_mg:
                nc.gpsimd.dma_start(
                    out=wb_s[:, goff : goff + gw], in_=wb[:, goff : goff + gw]
                )
            dps = dpsum.tile([128, GRP], F32)
            if USE_INJECT:
                for coff, cw in chunks:
                    nc.tensor.matmul(
                        dps[:, coff : coff + cw],
                        lhsT=ident,
                        rhs=t_s[:, coff : coff + cw],
                        start=True,
                        stop=False,
                    )
                for coff, cw in chunks:
                    nc.tensor.matmul(
                        dps[:, coff : coff + cw],
                        lhsT=lhsT_mg,
                        rhs=wb_s[:, goff + coff : goff + coff + cw],
                        start=False,
                        stop=True,
                    )
            else:
                for coff, cw in chunks:
                    nc.tensor.matmul(
                        dps[:, coff : coff + cw],
                        lhsT=lhsT_mg,
                        rhs=wb_s[:, goff + coff : goff + coff + cw],
                        start=True,
                        stop=True,
                    )
                # d = t + (-Xe @ W'), in place in PSUM
                nc.vector.tensor_add(dps[:, 0:gw], t_s[:, 0:gw], dps[:, 0:gw])
            s_s = spool.tile([128, GRP], F32)
            nc.scalar.square(s_s[:, 0:gw], dps[:, 0:gw])
            # transposed column reduce: out[c, 0] = sum_rows s[row, c]
            for j in range((gw + CCH - 1) // CCH):
                cw = min(CCH, gw - j * CCH)
                slot = mg * 256 + goff // CCH + j
                nc.tensor.matmul(
                    colsum[0:cw, slot : slot + 1],
                    lhsT=s_s[:, j * CCH : j * CCH + cw],
                    rhs=ones,
                    start=True,
                    stop=True,
                )
        first_mg = False

    # ---- phase 3: epilogue ----
    # combo columns: 0 = sse(mg0), 1 = sse(mg1), 2 = sum(logR) main,
    #                3 = sum(logR) remainder, 4 = kl_raw
    combo = small.tile([128, 5], F32)
    nc.vector.memset(combo, 0.0)

    prod = small.tile([128, NCC], F32)
    for mg in range(2):
        nc.vector.tensor_mul(prod, colsum[:, mg * 256 : mg * 256 + NCC], wfull[:, 0:NCC])
        nc.vector.reduce_sum(
            out=combo[:, mg : mg + 1], in_=prod, axis=mybir.AxisListType.X
        )

    nc.vector.reduce_sum(out=combo[0:N_FULL, 2:3], in_=lr37, axis=mybir.AxisListType.X)
    nc.vector.reduce_sum(out=combo[0:1, 3:4], in_=lrrem, axis=mybir.AxisListType.X)
    nc.vector.tensor_add(combo[:, 4:5], kl2[:, 0:1], kl2[:, 1:2])

    fps = smallps.tile([5, 1], F32, tag="sps")
    nc.tensor.matmul(fps, lhsT=combo, rhs=ones, start=True, stop=True)
    res = small.tile([5, 1], F32)
    nc.scalar.copy(res, fps)
    nc.sync.dma_start(out=out[:].rearrange("(p f) -> p f", f=1), in_=res)


_CACHED_NC = {}


def _get_nc(reps: int = 1):
    if reps not in _CACHED_NC:
        _CACHED_NC[reps] = build_nc(reps)
    return _CACHED_NC[reps]


def make_in_maps(mu_filtered, sigma_filtered, mu_pred, sigma_pred, target,
                 W_dec, b_dec, log_R, eps):
    tgt = np.asarray(target, dtype=np.float32).reshape(ROWS, D_OBS)
    wbf = np.concatenate(
        [np.asarray(W_dec, dtype=np.float32),
         np.asarray(b_dec, dtype=np.float32)[None, :]], axis=0
    )
    lr = np.asarray(log_R, dtype=np.float32)
    smalls = {
        "mu_f": np.ascontiguousarray(
            np.asarray(mu_filtered, dtype=np.float32).reshape(ROWS, LAT)),
        "sig_f": np.ascontiguousarray(
            np.asarray(sigma_filtered, dtype=np.float32).reshape(ROWS, 4 * Z)),
        "mu_p": np.ascontiguousarray(
            np.asarray(mu_pred, dtype=np.float32).reshape(ROWS, LAT)),
        "sig_p": np.ascontiguousarray(
            np.asarray(sigma_pred, dtype=np.float32).reshape(ROWS, 4 * Z)),
        "eps": np.ascontiguousarray(
            np.asarray(eps, dtype=np.float32).reshape(ROWS, LAT)),
    }
    in_maps = []
    for c in range(NCORES):
        sl = slice(c * DC, (c + 1) * DC)
        in_maps.append({
            **smalls,
            "tgt": np.ascontiguousarray(tgt[:, sl]),
            "wb": np.ascontiguousarray(wbf[:, sl]),
            "log_r": np.ascontiguousarray(lr[sl]),
        })
    return in_maps


def combine(results):
    sse = 0.0
    slr = 0.0
    for c in range(NCORES):
        v = results[c]["out"]
        sse += float(v[0]) + float(v[1])
        slr += float(v[2]) + float(v[3])
    klraw = float(results[0]["out"][4])
    n_tot = ROWS * D_OBS
    loss_integral = 0.5 * (
        n_tot * math.log(2.0 * math.pi) + 2.0 * ROWS * slr + sse
    ) / B
    loss_kl = 0.5 * (klraw - 2.0 * B * T * Z) / B
    return np.float32(loss_integral + loss_kl)


def kernel(mu_filtered, sigma_filtered, mu_pred, sigma_pred, target,
           W_dec, b_dec, log_R, eps):
    nc = _get_nc(1)
    in_maps = make_in_maps(mu_filtered, sigma_filtered, mu_pred, sigma_pred,
                           target, W_dec, b_dec, log_R, eps)
    res = run_bass_kernel_spmd(nc, in_maps, core_ids=list(range(NCORES)))
    return combine(res.results)


# revision 22
# speedup vs baseline: 501.3582x; 501.3582x over previous
"""Trainium2 Bass kernel for the DeepBayesianFilterBlockDiag loss.

Strategy (8-core SPMD, observation-axis sharded):
  - The 152064-dim observation axis is split into 8 shards of 19008 columns.
    Each core gets its shard of target [256,19008], W_dec||b_dec [65,19008],
    log_R [19008], plus the full (tiny) per-(b,t,z) tensors.
  - Per core:
      * compute Xe = [mu_f + chol(sigma_f) @ eps, 1] (tiny 2x2 algebra),
        transposed+negated into a [65,256] lhsT.
      * main loop over column chunks: PE "injects" the target into PSUM via an
        identity matmul, then accumulates -Xe @ W' on top, leaving
        d = t - rec in PSUM with zero DVE work.  ACT squares d into SBUF,
        PE reduces columns (ones-lhsT matmul) into a per-chunk row of a
        PSUM accumulator.
      * epilogue: weight column sums by exp(-2 log_R) (DVE fused
        tensor_tensor_reduce), reduce log_R, reduce the KL partials, and
        emit a [3] vector (sse, sum_logR, kl_raw).
  - Host combines the 8 partial vectors into the final scalar loss.
"""

import math

import numpy as np

import concourse.bass as bass
import concourse.mybir as mybir
import concourse.tile as tile
from concourse.bass_utils import run_bass_kernel_spmd
from concourse.masks import make_identity

F32 = mybir.dt.float32
BF16 = mybir.dt.bfloat16
AF = mybir.ActivationFunctionType
OP = mybir.AluOpType

B, T, Z, DIM = 4, 64, 32, 2
ROWS = B * T          # 256
LAT = Z * DIM         # 64
LATP = LAT + 1        # 65 (ones row folds in b_dec)
D_OBS = 152064
NCORES = 8
DC = D_OBS // NCORES  # 19008 columns per core
CH = 512              # psum-bank column chunk
GRP = 1024            # ACT / psum group (2 chunks)
N_FULL = DC // CH     # 37 full chunks
REM = DC - N_FULL * CH  # 64

CCH = 128             # colsum chunk (transposed-reduce matmul width)
MAX_DRAIN_WAITS = 1
USE_INJECT = True
ABLATE = set()  # perf-debug: subset of {"phase1","inject","mains","square","colsum","dma_t","dma_wb"}


def _layout(dc):
    groups = []
    off = 0
    while off < dc:
        g = []
        goff = off
        for _ in range(GRP // CH):
            w = min(CH, dc - off)
            if w <= 0:
                break
            g.append((off - goff, w))
            off += w
        groups.append((goff, g))
    n_full = dc // CH
    rem = dc - n_full * CH
    ncc = (dc + CCH - 1) // CCH
    return groups, n_full, rem, ncc


def _split_multi_waits(nc, max_waits=1):
    """walrus' per-instruction sync encoding only fits one wait; move extra
    waits emitted by Tile onto NOPs inserted just before the instruction on
    the same engine (same semantics: engine blocks on all of them in order).
    """
    k = 0
    for f in nc.m.functions:
        for blk in f.blocks:
            il = blk.instructions
            i = 0
            while i < len(il):
                inst = il[i]
                si = inst.sync_info
                if si is not None and len(si.on_wait) > max_waits:
                    waits = list(si.on_wait)
                    inst.sync_info = mybir.SyncInfo(
                        on_wait=waits[-max_waits:], on_update=list(si.on_update)
                    )
                    extra = waits[:-max_waits]
                    for j in range(0, len(extra), max_waits):
                        nop = mybir.InstEventSemaphore(
                            name=f"{inst.name}-w{k}",
                            engine=inst.engine,
                            sync_info=mybir.SyncInfo(
                                on_wait=extra[j : j + max_waits], on_update=[]
                            ),
                        )
                        k += 1
                        il.insert(i, nop)
                        i += 1
                i += 1


def _comp4(t, mg, idx):
    # [128, 2, 128] tile -> [128, 32] view of 2x2-block component idx
    return t[:, mg, :].rearrange("p (z k) -> p z k", k=4)[:, :, idx]


def _comp2(t, mg, idx):
    return t[:, mg, :].rearrange("p (z k) -> p z k", k=2)[:, :, idx]


def build_nc(reps: int = 1, dc: int = DC, split_waits: bool = True):
    nc = bass.Bass("TRN2")
    tgt = nc.dram_tensor("tgt", [ROWS, dc], F32, kind="ExternalInput")
    wb = nc.dram_tensor("wb", [LATP, dc], F32, kind="ExternalInput")
    lr = nc.dram_tensor("log_r", [dc], F32, kind="ExternalInput")
    muf = nc.dram_tensor("mu_f", [ROWS, LAT], F32, kind="ExternalInput")
    sgf = nc.dram_tensor("sig_f", [ROWS, 4 * Z], F32, kind="ExternalInput")
    mup = nc.dram_tensor("mu_p", [ROWS, LAT], F32, kind="ExternalInput")
    sgp = nc.dram_tensor("sig_p", [ROWS, 4 * Z], F32, kind="ExternalInput")
    eps = nc.dram_tensor("eps", [ROWS, LAT], F32, kind="ExternalInput")
    out = nc.dram_tensor("out", [5], F32, kind="ExternalOutput")

    with tile.TileContext(nc) as tc:
        with (
            tc.tile_pool(name="big", bufs=1) as big,
            tc.tile_pool(name="tp", bufs=6) as tpool,
            tc.tile_pool(name="sp", bufs=3) as spool,
            tc.tile_pool(name="small", bufs=1) as small,
            tc.tile_pool(name="dps", bufs=3, space="PSUM") as dpsum,
            tc.tile_pool(name="acc", bufs=1, space="PSUM") as accpsum,
            tc.tile_pool(name="smallps", bufs=1, space="PSUM") as smallps,
        ):
            if reps == 1:
                _body(nc, tc, big, tpool, spool, small, dpsum, accpsum, smallps,
                      tgt, wb, lr, muf, sgf, mup, sgp, eps, out, dc)
            else:
                with tc.For_i(0, reps, 1):
                    _body(nc, tc, big, tpool, spool, small, dpsum, accpsum,
                          smallps, tgt, wb, lr, muf, sgf, mup, sgp, eps, out, dc)
    if split_waits:
        # needed for the walrus/HW path; CoreSim wants the raw form
        _split_multi_waits(nc)
    return nc


def _body(nc, tc, big, tpool, spool, small, dpsum, accpsum, smallps,
          tgt, wb, lr, muf, sgf, mup, sgp, eps, out, dc=DC):
    GROUPS, N_FULL, REM, NCC = _layout(dc)
    DCL = dc
    ident = small.tile([128, 128], F32)
    make_identity(nc, ident)
    ones = small.tile([128, 1], F32)
    nc.vector.memset(ones, 1.0)
    ones_bf = small.tile([128, 1], BF16)
    nc.vector.memset(ones_bf, 1.0)

    # ---- small inputs ----
    sigf_s = small.tile([128, 2, 4 * Z], F32)
    sigp_s = small.tile([128, 2, 4 * Z], F32)
    muf_s = small.tile([128, 2, LAT], F32)
    mup_s = small.tile([128, 2, LAT], F32)
    eps_s = small.tile([128, 2, LAT], F32)
    for mg in range(2):
        rs = slice(mg * 128, (mg + 1) * 128)
        nc.sync.dma_start(out=sigf_s[:, mg, :], in_=sgf[rs, :])
        nc.sync.dma_start(out=sigp_s[:, mg, :], in_=sgp[rs, :])
        nc.sync.dma_start(out=muf_s[:, mg, :], in_=muf[rs, :])
        nc.sync.dma_start(out=mup_s[:, mg, :], in_=mup[rs, :])
        nc.sync.dma_start(out=eps_s[:, mg, :], in_=eps[rs, :])

    lr37 = small.tile([N_FULL, CH], F32)
    lrrem = small.tile([1, REM], F32)
    nc.sync.dma_start(
        out=lr37, in_=lr[0 : N_FULL * CH].rearrange("(p f) -> p f", f=CH)
    )
    nc.sync.dma_start(
        out=lrrem, in_=lr[N_FULL * CH : DCL].rearrange("(p f) -> p f", f=REM)
    )

    # ---- phase 1: Xe (cholesky sample) + KL, per 128-row group ----
    lhsT = small.tile([LATP, 256], F32)
    nc.vector.memset(lhsT[LAT:LATP, :], -1.0)
    kl2 = small.tile([128, 2], F32)

    if "phase1" in ABLATE:
        nc.vector.memset(lhsT, 0.01)
        nc.vector.memset(kl2, 0.0)
    for mg in range(2 if "phase1" not in ABLATE else 0):
        af = _comp4(sigf_s, mg, 0)
        bf = _comp4(sigf_s, mg, 1)
        cf = _comp4(sigf_s, mg, 2)
        df = _comp4(sigf_s, mg, 3)
        aq = _comp4(sigp_s, mg, 0)
        bq = _comp4(sigp_s, mg, 1)
        cq = _comp4(sigp_s, mg, 2)
        dq = _comp4(sigp_s, mg, 3)

        # cholesky: l11 = sqrt(a); l21 = c/l11; l22 = sqrt(d - l21^2)
        l11 = small.tile([128, Z], F32)
        nc.scalar.sqrt(l11, af)
        r11 = small.tile([128, Z], F32)
        nc.vector.reciprocal(r11, l11)
        l21 = small.tile([128, Z], F32)
        nc.vector.tensor_mul(l21, cf, r11)
        tmp0 = small.tile([128, Z], F32)
        nc.vector.tensor_mul(tmp0, l21, l21)
        nc.vector.tensor_sub(tmp0, df, tmp0)
        l22 = small.tile([128, Z], F32)
        nc.scalar.sqrt(l22, tmp0)

        e1 = _comp2(eps_s, mg, 0)
        e2 = _comp2(eps_s, mg, 1)
        m1 = _comp2(muf_s, mg, 0)
        m2 = _comp2(muf_s, mg, 1)

        xew = small.tile([128, LAT], F32)
        x1v = xew.rearrange("p (z k) -> p z k", k=2)[:, :, 0]
        x2v = xew.rearrange("p (z k) -> p z k", k=2)[:, :, 1]
        tA = small.tile([128, Z], F32)
        nc.vector.tensor_mul(tA, l11, e1)
        nc.vector.tensor_add(x1v, tA, m1)
        tB = small.tile([128, Z], F32)
        nc.vector.tensor_mul(tB, l21, e1)
        tC = small.tile([128, Z], F32)
        nc.vector.tensor_mul(tC, l22, e2)
        nc.vector.tensor_add(tB, tB, tC)
        nc.vector.tensor_add(x2v, tB, m2)

        tps = smallps.tile([LAT, 128], F32, tag="sps")
        nc.tensor.transpose(tps, xew, ident)
        nc.scalar.mul(lhsT[0:LAT, mg * 128 : (mg + 1) * 128], tps, -1.0)

        # KL pieces
        detq = small.tile([128, Z], F32)
        tD = small.tile([128, Z], F32)
        nc.vector.tensor_mul(detq, aq, dq)
        nc.vector.tensor_mul(tD, bq, cq)
        nc.vector.tensor_sub(detq, detq, tD)
        detp = small.tile([128, Z], F32)
        nc.vector.tensor_mul(detp, af, df)
        nc.vector.tensor_mul(tD, bf, cf)
        nc.vector.tensor_sub(detp, detp, tD)
        rdq = small.tile([128, Z], F32)
        nc.vector.reciprocal(rdq, detq)

        # trace numerator: dq*af - bq*bf - cq*cf + aq*df
        tn = small.tile([128, Z], F32)
        nc.vector.tensor_mul(tn, dq, af)
        nc.vector.tensor_mul(tD, aq, df)
        nc.vector.tensor_add(tn, tn, tD)
        nc.vector.tensor_mul(tD, bq, bf)
        nc.vector.tensor_sub(tn, tn, tD)
        nc.vector.tensor_mul(tD, cq, cf)
        nc.vector.tensor_sub(tn, tn, tD)

        # quad numerator: dq*d1^2 - (bq+cq)*d1*d2 + aq*d2^2
        p1 = _comp2(mup_s, mg, 0)
        p2 = _comp2(mup_s, mg, 1)
        d1 = small.tile([128, Z], F32)
        nc.vector.tensor_sub(d1, p1, m1)
        d2 = small.tile([128, Z], F32)
        nc.vector.tensor_sub(d2, p2, m2)
        qn = small.tile([128, Z], F32)
        nc.vector.tensor_mul(tD, d1, d1)
        nc.vector.tensor_mul(qn, dq, tD)
        nc.vector.tensor_mul(tD, d2, d2)
        nc.vector.tensor_mul(tD, aq, tD)
        nc.vector.tensor_add(qn, qn, tD)
        nc.vector.tensor_mul(tD, d1, d2)
        tE = small.tile([128, Z], F32)
        nc.vector.tensor_add(tE, bq, cq)
        nc.vector.tensor_mul(tD, tD, tE)
        nc.vector.tensor_sub(qn, qn, tD)

        klv = small.tile([128, Z], F32)
        nc.vector.tensor_add(klv, tn, qn)
        nc.vector.tensor_mul(klv, klv, rdq)
        # + ln(detq) - ln(detp)
        nc.scalar.activation(tD, detq, AF.Ln)
        nc.vector.tensor_add(klv, klv, tD)
        nc.scalar.activation(tD, detp, AF.Ln)
        nc.vector.tensor_sub(klv, klv, tD)
        nc.vector.reduce_sum(out=kl2[:, mg : mg + 1], in_=klv, axis=mybir.AxisListType.X)

    # w = exp(-2 log_R) (same ACT table set as Ln)
    w37 = small.tile([N_FULL, CH], F32)
    nc.scalar.activation(w37, lr37, AF.Exp, scale=-2.0)
    wrem = small.tile([1, REM], F32)
    nc.scalar.activation(wrem, lrrem, AF.Exp, scale=-2.0)

    # transpose w into [128, NCC]: wfull[p, cc] = w[cc*128 + p]
    wfull = small.tile([128, (N_FULL + 1) * (CH // CCH)], F32)  # [128, 152]
    nc.vector.memset(wfull, 0.0)
    wview = wfull.rearrange("p (r j) -> p r j", j=CH // CCH)  # [128, 38, 4]
    for j in range(CH // CCH):
        wtp = smallps.tile([128, N_FULL], F32, tag="sps")
        nc.tensor.transpose(wtp, w37[:, j * CCH : (j + 1) * CCH], ident[0:N_FULL, 0:N_FULL])
        nc.scalar.copy(wview[:, 0:N_FULL, j], wtp)
    wtr = smallps.tile([REM, 1], F32, tag="sps")
    nc.tensor.transpose(wtr, wrem, ident[0:1, 0:1])
    nc.scalar.copy(wfull[0:REM, NCC - 1 : NCC], wtr)

    # bf16 copies for the PE weight-heavy operands (fp32 LDWEIGHTS is 4x slow)
    lhsT_bf = small.tile([LATP, 256], BF16)
    nc.vector.tensor_copy(lhsT_bf, lhsT)

    # ---- W' (with b_dec row) resident in SBUF, loaded per-group ----
    wb_s = big.tile([LATP, DCL], F32)
    wb_bf = big.tile([LATP, DCL], BF16)

    # colsum bank: column-sums of squares land on partitions.
    # mg0 -> free slots [0, NCC), mg1 -> [256, 256+NCC)
    colsum = accpsum.tile([128, 512], F32)
    nc.vector.memset(colsum, 0.0)

    # ---- phase 2: main loop ----
    first_mg = True
    for mg in range(2):
        rs = slice(mg * 128, (mg + 1) * 128)
        lhsT_mg = lhsT[:, mg * 128 : (mg + 1) * 128]
        for goff, chunks in GROUPS:
            gw = sum(w for _, w in chunks)
            t_s = tpool.tile([128, GRP], F32)
            if "dma_t" not in ABLATE:
                nc.sync.dma_start(out=t_s[:, 0:gw], in_=tgt[rs, goff : goff + gw])
            if first_mg and "dma_wb" not in ABLATE:
                nc.sync.dma_start(
                    out=wb_s[:, goff : goff + gw], in_=wb[:, goff : goff + gw]
                )
                nc.vector.tensor_copy(
                    wb_bf[:, goff : goff + gw], wb_s[:, goff : goff + gw]
                )
            dps = dpsum.tile([128, GRP], F32)
            if "inject" in ABLATE or "mains" in ABLATE:
                if "inject" not in ABLATE:
                    for coff, cw in chunks:
                        nc.tensor.matmul(
                            dps[:, coff : coff + cw], lhsT=ident,
                            rhs=t_s[:, coff : coff + cw], start=True, stop=True)
                elif "mains" not in ABLATE:
                    for coff, cw in chunks:
                        nc.tensor.matmul(
                            dps[:, coff : coff + cw], lhsT=lhsT_mg,
                            rhs=wb_s[:, goff + coff : goff + coff + cw],
                            start=True, stop=True)
                else:
                    nc.vector.memset(dps[:, 0:gw], 0.0)
            elif USE_INJECT:
                for coff, cw in chunks:
                    nc.tensor.matmul(
                        dps[:, coff : coff + cw],
                        lhsT=ident,
                        rhs=t_s[:, coff : coff + cw],
                        start=True,
                        stop=False,
                    )
                for coff, cw in chunks:
                    nc.tensor.matmul(
                        dps[:, coff : coff + cw],
                        lhsT=lhsT_bf[:, mg * 128 : (mg + 1) * 128],
                        rhs=wb_bf[:, goff + coff : goff + coff + cw],
                        start=False,
                        stop=True,
                    )
            else:
                for coff, cw in chunks:
                    nc.tensor.matmul(
                        dps[:, coff : coff + cw],
                        lhsT=lhsT_mg,
                        rhs=wb_s[:, goff + coff : goff + coff + cw],
                        start=True,
                        stop=True,
                    )
                # d = t + (-Xe @ W'), in place in PSUM
                nc.vector.tensor_add(dps[:, 0:gw], t_s[:, 0:gw], dps[:, 0:gw])
            s_s = spool.tile([128, GRP], BF16)
            if "square" not in ABLATE:
                nc.scalar.square(s_s[:, 0:gw], dps[:, 0:gw])
            elif first_mg and goff == 0:
                nc.vector.memset(s_s, 0.0)
            # transposed column reduce: out[c, 0] = sum_rows s[row, c]
            for j in range((gw + CCH - 1) // CCH if "colsum" not in ABLATE else 0):
                cw = min(CCH, gw - j * CCH)
                slot = mg * 256 + goff // CCH + j
                nc.tensor.matmul(
                    colsum[0:cw, slot : slot + 1],
                    lhsT=s_s[:, j * CCH : j * CCH + cw],
                    rhs=ones_bf,
                    start=True,
                    stop=True,
                )
        first_mg = False

    # ---- phase 3: epilogue ----
    # combo columns: 0 = sse(mg0), 1 = sse(mg1), 2 = sum(logR) main,
    #                3 = sum(logR) remainder, 4 = kl_raw
    combo = small.tile([128, 5], F32)
    nc.vector.memset(combo, 0.0)

    prod = small.tile([128, NCC], F32)
    for mg in range(2):
        nc.vector.tensor_mul(prod, colsum[:, mg * 256 : mg * 256 + NCC], wfull[:, 0:NCC])
        nc.vector.reduce_sum(
            out=combo[:, mg : mg + 1], in_=prod, axis=mybir.AxisListType.X
        )

    nc.vector.reduce_sum(out=combo[0:N_FULL, 2:3], in_=lr37, axis=mybir.AxisListType.X)
    nc.vector.reduce_sum(out=combo[0:1, 3:4], in_=lrrem, axis=mybir.AxisListType.X)
    nc.vector.tensor_add(combo[:, 4:5], kl2[:, 0:1], kl2[:, 1:2])

    fps = smallps.tile([5, 1], F32, tag="sps")
    nc.tensor.matmul(fps, lhsT=combo, rhs=ones, start=True, stop=True)
    res = small.tile([5, 1], F32)
    nc.scalar.copy(res, fps)
    nc.sync.dma_start(out=out[:].rearrange("(p f) -> p f", f=1), in_=res)


_CACHED_NC = {}


def _get_nc(reps: int = 1):
    key = (reps, frozenset(ABLATE))
    if key not in _CACHED_NC:
        _CACHED_NC[key] = build_nc(reps)
    return _CACHED_NC[key]


def make_in_maps(mu_filtered, sigma_filtered, mu_pred, sigma_pred, target,
                 W_dec, b_dec, log_R, eps):
    tgt = np.asarray(target, dtype=np.float32).reshape(ROWS, D_OBS)
    wbf = np.concatenate(
        [np.asarray(W_dec, dtype=np.float32),
         np.asarray(b_dec, dtype=np.float32)[None, :]], axis=0
    )
    lr = np.asarray(log_R, dtype=np.float32)
    smalls = {
        "mu_f": np.ascontiguousarray(
            np.asarray(mu_filtered, dtype=np.float32).reshape(ROWS, LAT)),
        "sig_f": np.ascontiguousarray(
            np.asarray(sigma_filtered, dtype=np.float32).reshape(ROWS, 4 * Z)),
        "mu_p": np.ascontiguousarray(
            np.asarray(mu_pred, dtype=np.float32).reshape(ROWS, LAT)),
        "sig_p": np.ascontiguousarray(
            np.asarray(sigma_pred, dtype=np.float32).reshape(ROWS, 4 * Z)),
        "eps": np.ascontiguousarray(
            np.asarray(eps, dtype=np.float32).reshape(ROWS, LAT)),
    }
    in_maps = []
    for c in range(NCORES):
        sl = slice(c * DC, (c + 1) * DC)
        in_maps.append({
            **smalls,
            "tgt": np.ascontiguousarray(tgt[:, sl]),
            "wb": np.ascontiguousarray(wbf[:, sl]),
            "log_r": np.ascontiguousarray(lr[sl]),
        })
    return in_maps


def combine(results):
    sse = 0.0
    slr = 0.0
    for c in range(NCORES):
        v = results[c]["out"]
        sse += float(v[0]) + float(v[1])
        slr += float(v[2]) + float(v[3])
    klraw = float(results[0]["out"][4])
    n_tot = ROWS * D_OBS
    loss_integral = 0.5 * (
        n_tot * math.log(2.0 * math.pi) + 2.0 * ROWS * slr + sse
    ) / B
    loss_kl = 0.5 * (klraw - 2.0 * B * T * Z) / B
    return np.float32(loss_integral + loss_kl)


def kernel(mu_filtered, sigma_filtered, mu_pred, sigma_pred, target,
           W_dec, b_dec, log_R, eps):
    nc = _get_nc(1)
    in_maps = make_in_maps(mu_filtered, sigma_filtered, mu_pred, sigma_pred,
                           target, W_dec, b_dec, log_R, eps)
    res = run_bass_kernel_spmd(nc, in_maps, core_ids=list(range(NCORES)))
    return combine(res.results)
